# revision 1
# baseline (speedup 1.0000x reference)
"""MoE layer (8 experts, top-2, SwiGLU FFN) on 8 Trainium2 NeuronCores.

Strategy: expert parallelism. The gate is tiny (T x 8 matmul, 0.03% of the
FLOPs) and is computed on the host as part of input sharding; it decides the
all-to-all token dispatch. Core e runs expert e's SwiGLU FFN over the tokens
routed to it (padded to a fixed capacity C). The host scatters the gated
expert outputs back into the full (B, S, D) output.

Per-core device kernel (Bass/Tile, fp32):
  Phase A: x resident in SBUF; stream w_g/w_u once; psum accumulates over d;
           silu(g) * u -> hidden scratch in DRAM ([128, H/128, C] layout).
  Phase B: w_d resident in SBUF; stream hidden back per 128-token tile;
           psum accumulates over h; y written in natural (C, D) layout.
"""

import os

import numpy as np

import concourse.bass as bass  # noqa: F401  (bass registers mybir lowering)
import concourse.mybir as mybir
import concourse.tile as tile
from concourse import bacc
from concourse.bass_utils import run_bass_kernel_spmd

N_EXPERTS = 8
TOP_K = 2
D_MODEL = 1024
HIDDEN = 4096

F32 = mybir.dt.float32

_BUILD_CACHE = {}


def _build_expert_kernel(C, D=D_MODEL, H=HIDDEN, HG=512, TA=512):
    """Per-core expert FFN kernel. C (token capacity) must be a multiple of 128."""
    assert C % 128 == 0 and D % 128 == 0 and H % HG == 0 and HG % 128 == 0
    DO = D // 128
    HO = H // 128
    JG = HG // 128
    n_hg = H // HG
    t_chunks = [(t0, min(TA, C - t0)) for t0 in range(0, C, TA)]

    nc = bacc.Bacc("TRN2", target_bir_lowering=False, debug=False)

    xT = nc.dram_tensor("xT", [128, DO, C], F32, kind="ExternalInput")
    wg = nc.dram_tensor("wg", [128, DO, H], F32, kind="ExternalInput")
    wu = nc.dram_tensor("wu", [128, DO, H], F32, kind="ExternalInput")
    wd = nc.dram_tensor("wd", [128, HO, D], F32, kind="ExternalInput")
    y = nc.dram_tensor("y", [C, D], F32, kind="ExternalOutput")

    with tile.TileContext(nc) as tc:
        with tc.tile_pool(name="dram", bufs=1, space="DRAM") as dpool:
            hid = dpool.tile([128, HO, C], F32)

            # ---- Phase A: hidden = silu(x @ wg.T) * (x @ wu.T) ----
            with (
                tc.tile_pool(name="xpool", bufs=1) as xpool,
                tc.tile_pool(name="wA", bufs=2) as wpool,
                tc.tile_pool(name="hbuf", bufs=4) as hpool,
                tc.tile_pool(name="psA", bufs=2, space="PSUM") as psA,
            ):
                x_sb = xpool.tile([128, DO, C], F32)
                nc.sync.dma_start(x_sb[:], xT[:])

                for hg in range(n_hg):
                    wg_t = wpool.tile([128, DO, HG], F32, tag="wg")
                    nc.sync.dma_start(wg_t[:], wg[:, :, hg * HG : (hg + 1) * HG])
                    wu_t = wpool.tile([128, DO, HG], F32, tag="wu")
                    nc.sync.dma_start(wu_t[:], wu[:, :, hg * HG : (hg + 1) * HG])

                    for t0, tn in t_chunks:
                        for jj in range(JG):
                            j = hg * JG + jj
                            pg = psA.tile([128, TA], F32, tag="pg", name="pg")[:, :tn]
                            pu = psA.tile([128, TA], F32, tag="pu", name="pu")[:, :tn]
                            for do in range(DO):
                                nc.tensor.matmul(
                                    pg,
                                    lhsT=wg_t[:, do, jj * 128 : (jj + 1) * 128],
                                    rhs=x_sb[:, do, t0 : t0 + tn],
                                    start=(do == 0),
                                    stop=(do == DO - 1),
                                )
                            for do in range(DO):
                                nc.tensor.matmul(
                                    pu,
                                    lhsT=wu_t[:, do, jj * 128 : (jj + 1) * 128],
                                    rhs=x_sb[:, do, t0 : t0 + tn],
                                    start=(do == 0),
                                    stop=(do == DO - 1),
                                )
                            # silu(pg) * pu via sigmoid
                            sg = hpool.tile([128, TA], F32, tag="sg", name="sg")[:, :tn]
                            nc.scalar.activation(
                                sg, pg, mybir.ActivationFunctionType.Sigmoid
                            )
                            hs = hpool.tile([128, TA], F32, tag="h", name="hs")[:, :tn]
                            nc.vector.tensor_mul(hs, sg, pg)
                            nc.vector.tensor_mul(hs, hs, pu)
                            nc.sync.dma_start(hid[:, j, t0 : t0 + tn], hs)

            # ---- Phase B: y = hidden @ wd.T ----
            with (
                tc.tile_pool(name="wdpool", bufs=1) as wdpool,
                tc.tile_pool(name="hload", bufs=2) as hlpool,
                tc.tile_pool(name="ybuf", bufs=3) as ypool,
                tc.tile_pool(name="psB", bufs=4, space="PSUM") as psB,
            ):
                wd_sb = wdpool.tile([128, HO, D], F32)
                nc.sync.dma_start(wd_sb[:], wd[:])

                d_chunks = [(d0, min(512, D - d0)) for d0 in range(0, D, 512)]
                for tb in range(C // 128):
                    hid_t = hlpool.tile([128, HO, 128], F32)
                    nc.sync.dma_start(hid_t[:], hid[:, :, tb * 128 : (tb + 1) * 128])
                    y_sb = ypool.tile([128, D], F32)
                    for d0, dn in d_chunks:
                        py = psB.tile([128, 512], F32, tag="py", name="py")[:, :dn]
                        for j in range(HO):
                            nc.tensor.matmul(
                                py,
                                lhsT=hid_t[:, j, :],
                                rhs=wd_sb[:, j, d0 : d0 + dn],
                                start=(j == 0),
                                stop=(j == HO - 1),
                            )
                        nc.any.tensor_copy(y_sb[:, d0 : d0 + dn], py)
                    nc.sync.dma_start(y[tb * 128 : (tb + 1) * 128, :], y_sb)

    nc.compile()
    return nc


def _get_kernel(C):
    if C not in _BUILD_CACHE:
        _BUILD_CACHE[C] = _build_expert_kernel(C)
    return _BUILD_CACHE[C]


def _pack_core_inputs(x_pad, w_g_e, w_u_e, w_d_e):
    C, D = x_pad.shape
    H = w_g_e.shape[0]
    return {
        "xT": np.ascontiguousarray(x_pad.reshape(C, D // 128, 128).transpose(2, 1, 0)),
        "wg": np.ascontiguousarray(
            w_g_e.reshape(H, D // 128, 128).transpose(2, 1, 0)
        ),
        "wu": np.ascontiguousarray(
            w_u_e.reshape(H, D // 128, 128).transpose(2, 1, 0)
        ),
        "wd": np.ascontiguousarray(
            w_d_e.reshape(D, H // 128, 128).transpose(2, 1, 0)
        ),
    }


def kernel(x, w_gate, w_noise, w_g, w_u, w_d, _collect_perf=None):
    x = np.asarray(x, dtype=np.float32)
    w_gate = np.asarray(w_gate, dtype=np.float32)
    w_g = np.asarray(w_g, dtype=np.float32)
    w_u = np.asarray(w_u, dtype=np.float32)
    w_d = np.asarray(w_d, dtype=np.float32)

    B, S, D = x.shape
    T = B * S
    xf = x.reshape(T, D)

    # ---- Gating (host; decides the token dispatch) ----
    logits = xf @ w_gate.T  # (T, E) fp32
    rows = np.arange(T)
    top1 = np.argmax(logits, axis=1)
    l1 = logits[rows, top1]
    masked = logits.copy()
    masked[rows, top1] = -np.inf
    top2 = np.argmax(masked, axis=1)
    l2 = logits[rows, top2]
    # softmax over the two top logits (l1 >= l2)
    e2 = np.exp((l2 - l1).astype(np.float64))
    g1 = (1.0 / (1.0 + e2)).astype(np.float32)
    g2 = (e2 / (1.0 + e2)).astype(np.float32)

    # aux loss: E * sum(mean(gates) * mean(softmax(logits)))
    lg64 = logits.astype(np.float64)
    lg64 -= lg64.max(axis=1, keepdims=True)
    p = np.exp(lg64)
    p /= p.sum(axis=1, keepdims=True)
    P = p.mean(axis=0)
    f = np.zeros(N_EXPERTS, dtype=np.float64)
    np.add.at(f, top1, g1.astype(np.float64))
    np.add.at(f, top2, g2.astype(np.float64))
    f /= T
    aux_loss = np.float32(N_EXPERTS * np.sum(f * P))

    # ---- Token dispatch (host all-to-all) ----
    idxs, gates_e = [], []
    for e in range(N_EXPERTS):
        m1 = top1 == e
        m2 = top2 == e
        idx = np.nonzero(m1 | m2)[0]
        ge = np.where(m1[idx], g1[idx], g2[idx]).astype(np.float32)
        idxs.append(idx)
        gates_e.append(ge)

    max_cnt = max(len(i) for i in idxs)
    C = max(128, ((max_cnt + 127) // 128) * 128)
    nc = _get_kernel(C)

    in_maps = []
    for e in range(N_EXPERTS):
        idx = idxs[e]
        x_pad = np.zeros((C, D), np.float32)
        x_pad[: len(idx)] = xf[idx]
        in_maps.append(_pack_core_inputs(x_pad, w_g[e], w_u[e], w_d[e]))

    trace = bool(int(os.environ.get("MOE_KERNEL_TRACE", "0")))
    res = run_bass_kernel_spmd(
        nc,
        in_maps,
        list(range(N_EXPERTS)),
        trace=trace,
        trace_cores=list(range(N_EXPERTS)) if trace else None,
    )
    if _collect_perf is not None:
        _collect_perf.append(res)

    # ---- Combine (host scatter-add of gated expert outputs) ----
    out = np.zeros((T, D), np.float32)
    for e in range(N_EXPERTS):
        idx = idxs[e]
        if len(idx) == 0:
            continue
        ye = np.asarray(res.results[e]["y"])[: len(idx)]
        out[idx] += gates_e[e][:, None] * ye

    return out.reshape(B, S, D), aux_loss


# revision 2
# speedup vs baseline: 3.5641x; 3.5641x over previous
"""MoE layer (8 experts, top-2, SwiGLU FFN) on 8 Trainium2 NeuronCores.

Strategy: expert parallelism. The gate is tiny (T x 8 matmul, 0.03% of the
FLOPs) and is computed on the host as part of input sharding; it decides the
all-to-all token dispatch. Core e runs expert e's SwiGLU FFN over the tokens
routed to it (padded to a fixed capacity C). The host scatters the gated
expert outputs back into the full (B, S, D) output.

Per-core device kernel (Bass/Tile, fp32):
  Phase A: x resident in SBUF; stream w_g/w_u once; psum accumulates over d;
           silu(g) * u -> hidden scratch in DRAM ([128, H/128, C] layout).
  Phase B: w_d resident in SBUF; stream hidden back per 128-token tile;
           psum accumulates over h; y written in natural (C, D) layout.
"""

import os

import numpy as np

import concourse.bass as bass  # noqa: F401  (bass registers mybir lowering)
import concourse.mybir as mybir
import concourse.tile as tile
from concourse import bacc
from concourse.bass_utils import run_bass_kernel_spmd

N_EXPERTS = 8
TOP_K = 2
D_MODEL = 1024
HIDDEN = 4096

F32 = mybir.dt.float32
BF16 = mybir.dt.bfloat16

_BUILD_CACHE = {}


def _build_expert_kernel(C, D=D_MODEL, H=HIDDEN, HG=512, TA=512):
    """Per-core expert FFN kernel. C (token capacity) must be a multiple of 128."""
    assert C % 128 == 0 and D % 128 == 0 and H % HG == 0 and HG % 128 == 0
    DO = D // 128
    HO = H // 128
    JG = HG // 128
    n_hg = H // HG
    t_chunks = [(t0, min(TA, C - t0)) for t0 in range(0, C, TA)]

    nc = bacc.Bacc("TRN2", target_bir_lowering=False, debug=False)

    xT = nc.dram_tensor("xT", [128, DO, C], BF16, kind="ExternalInput")
    wg = nc.dram_tensor("wg", [128, DO, H], BF16, kind="ExternalInput")
    wu = nc.dram_tensor("wu", [128, DO, H], BF16, kind="ExternalInput")
    wd = nc.dram_tensor("wd", [128, HO, D], BF16, kind="ExternalInput")
    y = nc.dram_tensor("y", [C, D], F32, kind="ExternalOutput")

    with tile.TileContext(nc) as tc:
        with tc.tile_pool(name="dram", bufs=1, space="DRAM") as dpool:
            hid = dpool.tile([128, HO, C], BF16)

            # ---- Phase A: hidden = silu(x @ wg.T) * (x @ wu.T) ----
            with (
                tc.tile_pool(name="xpool", bufs=1) as xpool,
                tc.tile_pool(name="wA", bufs=2) as wpool,
                tc.tile_pool(name="hbuf", bufs=4) as hpool,
                tc.tile_pool(name="psA", bufs=2, space="PSUM") as psA,
            ):
                x_sb = xpool.tile([128, DO, C], BF16)
                nc.sync.dma_start(x_sb[:], xT[:])

                for hg in range(n_hg):
                    wg_t = wpool.tile([128, DO, HG], BF16, tag="wg")
                    nc.sync.dma_start(wg_t[:], wg[:, :, hg * HG : (hg + 1) * HG])
                    wu_t = wpool.tile([128, DO, HG], BF16, tag="wu")
                    nc.sync.dma_start(wu_t[:], wu[:, :, hg * HG : (hg + 1) * HG])

                    for t0, tn in t_chunks:
                        for jj in range(JG):
                            j = hg * JG + jj
                            pg = psA.tile([128, TA], F32, tag="pg", name="pg")[:, :tn]
                            pu = psA.tile([128, TA], F32, tag="pu", name="pu")[:, :tn]
                            for do in range(DO):
                                nc.tensor.matmul(
                                    pg,
                                    lhsT=wg_t[:, do, jj * 128 : (jj + 1) * 128],
                                    rhs=x_sb[:, do, t0 : t0 + tn],
                                    start=(do == 0),
                                    stop=(do == DO - 1),
                                )
                            for do in range(DO):
                                nc.tensor.matmul(
                                    pu,
                                    lhsT=wu_t[:, do, jj * 128 : (jj + 1) * 128],
                                    rhs=x_sb[:, do, t0 : t0 + tn],
                                    start=(do == 0),
                                    stop=(do == DO - 1),
                                )
                            # silu(pg) * pu via sigmoid
                            sg = hpool.tile([128, TA], F32, tag="sg", name="sg")[:, :tn]
                            nc.scalar.activation(
                                sg, pg, mybir.ActivationFunctionType.Sigmoid
                            )
                            t1 = hpool.tile([128, TA], F32, tag="t1", name="t1")[:, :tn]
                            nc.vector.tensor_mul(t1, sg, pg)
                            hs = hpool.tile([128, TA], BF16, tag="h", name="hs")[:, :tn]
                            nc.vector.tensor_mul(hs, t1, pu)
                            nc.sync.dma_start(hid[:, j, t0 : t0 + tn], hs)

            # ---- Phase B: y = hidden @ wd.T ----
            with (
                tc.tile_pool(name="wdpool", bufs=1) as wdpool,
                tc.tile_pool(name="hload", bufs=2) as hlpool,
                tc.tile_pool(name="ybuf", bufs=3) as ypool,
                tc.tile_pool(name="psB", bufs=4, space="PSUM") as psB,
            ):
                wd_sb = wdpool.tile([128, HO, D], BF16)
                nc.sync.dma_start(wd_sb[:], wd[:])

                d_chunks = [(d0, min(512, D - d0)) for d0 in range(0, D, 512)]
                for tb in range(C // 128):
                    hid_t = hlpool.tile([128, HO, 128], BF16)
                    nc.sync.dma_start(hid_t[:], hid[:, :, tb * 128 : (tb + 1) * 128])
                    y_sb = ypool.tile([128, D], F32)
                    for d0, dn in d_chunks:
                        py = psB.tile([128, 512], F32, tag="py", name="py")[:, :dn]
                        for j in range(HO):
                            nc.tensor.matmul(
                                py,
                                lhsT=hid_t[:, j, :],
                                rhs=wd_sb[:, j, d0 : d0 + dn],
                                start=(j == 0),
                                stop=(j == HO - 1),
                            )
                        nc.any.tensor_copy(y_sb[:, d0 : d0 + dn], py)
                    nc.sync.dma_start(y[tb * 128 : (tb + 1) * 128, :], y_sb)

    nc.compile()
    return nc


def _get_kernel(C):
    if C not in _BUILD_CACHE:
        _BUILD_CACHE[C] = _build_expert_kernel(C)
    return _BUILD_CACHE[C]


def _pack_core_inputs(x_pad, w_g_e, w_u_e, w_d_e):
    import ml_dtypes

    bf16 = ml_dtypes.bfloat16
    x_pad = x_pad.astype(bf16)
    w_g_e = w_g_e.astype(bf16)
    w_u_e = w_u_e.astype(bf16)
    w_d_e = w_d_e.astype(bf16)
    C, D = x_pad.shape
    H = w_g_e.shape[0]
    return {
        "xT": np.ascontiguousarray(x_pad.reshape(C, D // 128, 128).transpose(2, 1, 0)),
        "wg": np.ascontiguousarray(
            w_g_e.reshape(H, D // 128, 128).transpose(2, 1, 0)
        ),
        "wu": np.ascontiguousarray(
            w_u_e.reshape(H, D // 128, 128).transpose(2, 1, 0)
        ),
        "wd": np.ascontiguousarray(
            w_d_e.reshape(D, H // 128, 128).transpose(2, 1, 0)
        ),
    }


def kernel(x, w_gate, w_noise, w_g, w_u, w_d, _collect_perf=None):
    x = np.asarray(x, dtype=np.float32)
    w_gate = np.asarray(w_gate, dtype=np.float32)
    w_g = np.asarray(w_g, dtype=np.float32)
    w_u = np.asarray(w_u, dtype=np.float32)
    w_d = np.asarray(w_d, dtype=np.float32)

    B, S, D = x.shape
    T = B * S
    xf = x.reshape(T, D)

    # ---- Gating (host; decides the token dispatch) ----
    logits = xf @ w_gate.T  # (T, E) fp32
    rows = np.arange(T)
    top1 = np.argmax(logits, axis=1)
    l1 = logits[rows, top1]
    masked = logits.copy()
    masked[rows, top1] = -np.inf
    top2 = np.argmax(masked, axis=1)
    l2 = logits[rows, top2]
    # softmax over the two top logits (l1 >= l2)
    e2 = np.exp((l2 - l1).astype(np.float64))
    g1 = (1.0 / (1.0 + e2)).astype(np.float32)
    g2 = (e2 / (1.0 + e2)).astype(np.float32)

    # aux loss: E * sum(mean(gates) * mean(softmax(logits)))
    lg64 = logits.astype(np.float64)
    lg64 -= lg64.max(axis=1, keepdims=True)
    p = np.exp(lg64)
    p /= p.sum(axis=1, keepdims=True)
    P = p.mean(axis=0)
    f = np.zeros(N_EXPERTS, dtype=np.float64)
    np.add.at(f, top1, g1.astype(np.float64))
    np.add.at(f, top2, g2.astype(np.float64))
    f /= T
    aux_loss = np.float32(N_EXPERTS * np.sum(f * P))

    # ---- Token dispatch (host all-to-all) ----
    idxs, gates_e = [], []
    for e in range(N_EXPERTS):
        m1 = top1 == e
        m2 = top2 == e
        idx = np.nonzero(m1 | m2)[0]
        ge = np.where(m1[idx], g1[idx], g2[idx]).astype(np.float32)
        idxs.append(idx)
        gates_e.append(ge)

    max_cnt = max(len(i) for i in idxs)
    C = max(128, ((max_cnt + 127) // 128) * 128)
    nc = _get_kernel(C)

    in_maps = []
    for e in range(N_EXPERTS):
        idx = idxs[e]
        x_pad = np.zeros((C, D), np.float32)
        x_pad[: len(idx)] = xf[idx]
        in_maps.append(_pack_core_inputs(x_pad, w_g[e], w_u[e], w_d[e]))

    trace = bool(int(os.environ.get("MOE_KERNEL_TRACE", "0")))
    res = run_bass_kernel_spmd(
        nc,
        in_maps,
        list(range(N_EXPERTS)),
        trace=trace,
        trace_cores=list(range(N_EXPERTS)) if trace else None,
    )
    if _collect_perf is not None:
        _collect_perf.append(res)

    # ---- Combine (host scatter-add of gated expert outputs) ----
    out = np.zeros((T, D), np.float32)
    for e in range(N_EXPERTS):
        idx = idxs[e]
        if len(idx) == 0:
            continue
        ye = np.asarray(res.results[e]["y"])[: len(idx)]
        out[idx] += gates_e[e][:, None] * ye

    return out.reshape(B, S, D), aux_loss


# revision 3
# speedup vs baseline: 3.8524x; 1.0809x over previous
"""MoE layer (8 experts, top-2, SwiGLU FFN) on 8 Trainium2 NeuronCores.

Strategy: expert parallelism. The gate is tiny (T x 8 matmul, 0.03% of the
FLOPs) and is computed on the host as part of input sharding; it decides the
all-to-all token dispatch. Core e runs expert e's SwiGLU FFN over the tokens
routed to it (padded to a fixed capacity C). The host scatters the gated
expert outputs back into the full (B, S, D) output.

Per-core device kernel (Bass/Tile, fp32):
  Phase A: x resident in SBUF; stream w_g/w_u once; psum accumulates over d;
           silu(g) * u -> hidden scratch in DRAM ([128, H/128, C] layout).
  Phase B: w_d resident in SBUF; stream hidden back per 128-token tile;
           psum accumulates over h; y written in natural (C, D) layout.
"""

import os

import numpy as np

import concourse.bass as bass  # noqa: F401  (bass registers mybir lowering)
import concourse.mybir as mybir
import concourse.tile as tile
from concourse import bacc
from concourse.bass_utils import run_bass_kernel_spmd

N_EXPERTS = 8
TOP_K = 2
D_MODEL = 1024
HIDDEN = 4096

F32 = mybir.dt.float32
BF16 = mybir.dt.bfloat16

_BUILD_CACHE = {}


def _build_expert_kernel(C, D=D_MODEL, H=HIDDEN, HG=512, TA=512):
    """Per-core expert FFN kernel. C (token capacity) must be a multiple of 128."""
    assert C % 128 == 0 and D % 128 == 0 and H % HG == 0 and HG % 128 == 0
    DO = D // 128
    HO = H // 128
    JG = HG // 128
    n_hg = H // HG
    t_chunks = [(t0, min(TA, C - t0)) for t0 in range(0, C, TA)]

    nc = bacc.Bacc("TRN2", target_bir_lowering=False, debug=False)

    xT = nc.dram_tensor("xT", [128, DO, C], BF16, kind="ExternalInput")
    wg = nc.dram_tensor("wg", [128, DO, H], BF16, kind="ExternalInput")
    wu = nc.dram_tensor("wu", [128, DO, H], BF16, kind="ExternalInput")
    wd = nc.dram_tensor("wd", [128, HO, D], BF16, kind="ExternalInput")
    y = nc.dram_tensor("y", [C, D], F32, kind="ExternalOutput")

    with tile.TileContext(nc) as tc:
        with (
            tc.tile_pool(name="hidpool", bufs=1) as hidpool,
            tc.tile_pool(name="wdpool", bufs=2) as wdpool,
            tc.tile_pool(name="ybuf", bufs=3) as ypool,
            tc.tile_pool(name="psB", bufs=3, space="PSUM") as psB,
        ):
            # hidden stays resident in SBUF (bf16: H/128 * C * 2 bytes/partition)
            hid_sb = hidpool.tile([128, HO, C], BF16)

            # ---- Phase A: hidden = silu(x @ wg.T) * (x @ wu.T) ----
            with (
                tc.tile_pool(name="xpool", bufs=1) as xpool,
                tc.tile_pool(name="wA", bufs=2) as wpool,
                tc.tile_pool(name="hbuf", bufs=4) as hpool,
                tc.tile_pool(name="psA", bufs=2, space="PSUM") as psA,
            ):
                x_sb = xpool.tile([128, DO, C], BF16)
                nc.sync.dma_start(x_sb[:], xT[:])

                for hg in range(n_hg):
                    wg_t = wpool.tile([128, DO, HG], BF16, tag="wg")
                    nc.sync.dma_start(wg_t[:], wg[:, :, hg * HG : (hg + 1) * HG])
                    wu_t = wpool.tile([128, DO, HG], BF16, tag="wu")
                    nc.sync.dma_start(wu_t[:], wu[:, :, hg * HG : (hg + 1) * HG])

                    for t0, tn in t_chunks:
                        for jj in range(JG):
                            j = hg * JG + jj
                            pg = psA.tile([128, TA], F32, tag="pg", name="pg")[:, :tn]
                            pu = psA.tile([128, TA], F32, tag="pu", name="pu")[:, :tn]
                            for do in range(DO):
                                nc.tensor.matmul(
                                    pg,
                                    lhsT=wg_t[:, do, jj * 128 : (jj + 1) * 128],
                                    rhs=x_sb[:, do, t0 : t0 + tn],
                                    start=(do == 0),
                                    stop=(do == DO - 1),
                                )
                            for do in range(DO):
                                nc.tensor.matmul(
                                    pu,
                                    lhsT=wu_t[:, do, jj * 128 : (jj + 1) * 128],
                                    rhs=x_sb[:, do, t0 : t0 + tn],
                                    start=(do == 0),
                                    stop=(do == DO - 1),
                                )
                            # silu(pg) * pu via sigmoid, written straight into hid_sb
                            sg = hpool.tile([128, TA], F32, tag="sg", name="sg")[:, :tn]
                            nc.scalar.activation(
                                sg, pg, mybir.ActivationFunctionType.Sigmoid
                            )
                            t1 = hpool.tile([128, TA], F32, tag="t1", name="t1")[:, :tn]
                            nc.vector.tensor_mul(t1, sg, pg)
                            nc.vector.tensor_mul(hid_sb[:, j, t0 : t0 + tn], t1, pu)

            # ---- Phase B: y = hidden @ wd.T (hidden read from SBUF) ----
            DQ = 256  # d-chunk width; wd streamed once in DQ-wide quarters
            for d0 in range(0, D, DQ):
                wdh = wdpool.tile([128, HO, DQ], BF16, tag="wdh", name="wdh")
                nc.sync.dma_start(wdh[:], wd[:, :, d0 : d0 + DQ])
                for tb in range(C // 128):
                    py = psB.tile([128, DQ], F32, tag="py", name="py")
                    for j in range(HO):
                        nc.tensor.matmul(
                            py,
                            lhsT=hid_sb[:, j, tb * 128 : (tb + 1) * 128],
                            rhs=wdh[:, j, :],
                            start=(j == 0),
                            stop=(j == HO - 1),
                        )
                    y_sb = ypool.tile([128, DQ], F32, name="y_sb")
                    nc.any.tensor_copy(y_sb[:], py)
                    nc.sync.dma_start(y[tb * 128 : (tb + 1) * 128, d0 : d0 + DQ], y_sb)

    nc.compile()
    return nc


def _get_kernel(C):
    if C not in _BUILD_CACHE:
        _BUILD_CACHE[C] = _build_expert_kernel(C)
    return _BUILD_CACHE[C]


def _pack_core_inputs(x_pad, w_g_e, w_u_e, w_d_e):
    import ml_dtypes

    bf16 = ml_dtypes.bfloat16
    x_pad = x_pad.astype(bf16)
    w_g_e = w_g_e.astype(bf16)
    w_u_e = w_u_e.astype(bf16)
    w_d_e = w_d_e.astype(bf16)
    C, D = x_pad.shape
    H = w_g_e.shape[0]
    return {
        "xT": np.ascontiguousarray(x_pad.reshape(C, D // 128, 128).transpose(2, 1, 0)),
        "wg": np.ascontiguousarray(
            w_g_e.reshape(H, D // 128, 128).transpose(2, 1, 0)
        ),
        "wu": np.ascontiguousarray(
            w_u_e.reshape(H, D // 128, 128).transpose(2, 1, 0)
        ),
        "wd": np.ascontiguousarray(
            w_d_e.reshape(D, H // 128, 128).transpose(2, 1, 0)
        ),
    }


def kernel(x, w_gate, w_noise, w_g, w_u, w_d, _collect_perf=None):
    x = np.asarray(x, dtype=np.float32)
    w_gate = np.asarray(w_gate, dtype=np.float32)
    w_g = np.asarray(w_g, dtype=np.float32)
    w_u = np.asarray(w_u, dtype=np.float32)
    w_d = np.asarray(w_d, dtype=np.float32)

    B, S, D = x.shape
    T = B * S
    xf = x.reshape(T, D)

    # ---- Gating (host; decides the token dispatch) ----
    logits = xf @ w_gate.T  # (T, E) fp32
    rows = np.arange(T)
    top1 = np.argmax(logits, axis=1)
    l1 = logits[rows, top1]
    masked = logits.copy()
    masked[rows, top1] = -np.inf
    top2 = np.argmax(masked, axis=1)
    l2 = logits[rows, top2]
    # softmax over the two top logits (l1 >= l2)
    e2 = np.exp((l2 - l1).astype(np.float64))
    g1 = (1.0 / (1.0 + e2)).astype(np.float32)
    g2 = (e2 / (1.0 + e2)).astype(np.float32)

    # aux loss: E * sum(mean(gates) * mean(softmax(logits)))
    lg64 = logits.astype(np.float64)
    lg64 -= lg64.max(axis=1, keepdims=True)
    p = np.exp(lg64)
    p /= p.sum(axis=1, keepdims=True)
    P = p.mean(axis=0)
    f = np.zeros(N_EXPERTS, dtype=np.float64)
    np.add.at(f, top1, g1.astype(np.float64))
    np.add.at(f, top2, g2.astype(np.float64))
    f /= T
    aux_loss = np.float32(N_EXPERTS * np.sum(f * P))

    # ---- Token dispatch (host all-to-all) ----
    idxs, gates_e = [], []
    for e in range(N_EXPERTS):
        m1 = top1 == e
        m2 = top2 == e
        idx = np.nonzero(m1 | m2)[0]
        ge = np.where(m1[idx], g1[idx], g2[idx]).astype(np.float32)
        idxs.append(idx)
        gates_e.append(ge)

    max_cnt = max(len(i) for i in idxs)
    C = max(128, ((max_cnt + 127) // 128) * 128)
    nc = _get_kernel(C)

    in_maps = []
    for e in range(N_EXPERTS):
        idx = idxs[e]
        x_pad = np.zeros((C, D), np.float32)
        x_pad[: len(idx)] = xf[idx]
        in_maps.append(_pack_core_inputs(x_pad, w_g[e], w_u[e], w_d[e]))

    trace = bool(int(os.environ.get("MOE_KERNEL_TRACE", "0")))
    res = run_bass_kernel_spmd(
        nc,
        in_maps,
        list(range(N_EXPERTS)),
        trace=trace,
        trace_cores=list(range(N_EXPERTS)) if trace else None,
    )
    if _collect_perf is not None:
        _collect_perf.append(res)

    # ---- Combine (host scatter-add of gated expert outputs) ----
    out = np.zeros((T, D), np.float32)
    for e in range(N_EXPERTS):
        idx = idxs[e]
        if len(idx) == 0:
            continue
        ye = np.asarray(res.results[e]["y"])[: len(idx)]
        out[idx] += gates_e[e][:, None] * ye

    return out.reshape(B, S, D), aux_loss


# revision 4
# speedup vs baseline: 3.9340x; 1.0212x over previous
"""MoE layer (8 experts, top-2, SwiGLU FFN) on 8 Trainium2 NeuronCores.

Strategy: expert parallelism. The gate is tiny (T x 8 matmul, 0.03% of the
FLOPs) and is computed on the host as part of input sharding; it decides the
all-to-all token dispatch. Core e runs expert e's SwiGLU FFN over the tokens
routed to it (padded to a fixed capacity C). The host scatters the gated
expert outputs back into the full (B, S, D) output.

Per-core device kernel (Bass/Tile, fp32):
  Phase A: x resident in SBUF; stream w_g/w_u once; psum accumulates over d;
           silu(g) * u -> hidden scratch in DRAM ([128, H/128, C] layout).
  Phase B: w_d resident in SBUF; stream hidden back per 128-token tile;
           psum accumulates over h; y written in natural (C, D) layout.
"""

import os

import numpy as np

import concourse.bass as bass  # noqa: F401  (bass registers mybir lowering)
import concourse.mybir as mybir
import concourse.tile as tile
from concourse import bacc
from concourse.bass_utils import run_bass_kernel_spmd

N_EXPERTS = 8
TOP_K = 2
D_MODEL = 1024
HIDDEN = 4096

F32 = mybir.dt.float32
BF16 = mybir.dt.bfloat16

_BUILD_CACHE = {}


def _build_expert_kernel(C, D=D_MODEL, H=HIDDEN, HG=512, TA=512):
    """Per-core expert FFN kernel. C (token capacity) must be a multiple of 128."""
    assert C % 128 == 0 and D % 128 == 0 and H % HG == 0 and HG % 128 == 0
    DO = D // 128
    HO = H // 128
    JG = HG // 128
    n_hg = H // HG
    t_chunks = [(t0, min(TA, C - t0)) for t0 in range(0, C, TA)]

    nc = bacc.Bacc("TRN2", target_bir_lowering=False, debug=False)

    xT = nc.dram_tensor("xT", [128, DO, C], BF16, kind="ExternalInput")
    wg = nc.dram_tensor("wg", [128, DO, H], BF16, kind="ExternalInput")
    wu = nc.dram_tensor("wu", [128, DO, H], BF16, kind="ExternalInput")
    wd = nc.dram_tensor("wd", [128, HO, D], BF16, kind="ExternalInput")
    y = nc.dram_tensor("y", [C, D], F32, kind="ExternalOutput")

    with tile.TileContext(nc) as tc:
        with (
            tc.tile_pool(name="hidpool", bufs=1) as hidpool,
            tc.tile_pool(name="wdpool", bufs=2) as wdpool,
            tc.tile_pool(name="ybuf", bufs=3) as ypool,
            tc.tile_pool(name="psB", bufs=3, space="PSUM") as psB,
        ):
            # hidden stays resident in SBUF (bf16: H/128 * C * 2 bytes/partition)
            hid_sb = hidpool.tile([128, HO, C], BF16)

            # ---- Phase A: hidden = silu(x @ wg.T) * (x @ wu.T) ----
            with (
                tc.tile_pool(name="xpool", bufs=1) as xpool,
                tc.tile_pool(name="wA", bufs=2) as wpool,
                tc.tile_pool(name="hbuf", bufs=4) as hpool,
                tc.tile_pool(name="psA", bufs=2, space="PSUM") as psA,
            ):
                x_sb = xpool.tile([128, DO, C], BF16)
                for do in range(DO):
                    nc.sync.dma_start(x_sb[:, do, :], xT[:, do, :])

                for hg in range(n_hg):
                    wg_t = wpool.tile([128, DO, HG], BF16, tag="wg")
                    wu_t = wpool.tile([128, DO, HG], BF16, tag="wu")
                    for do in range(DO):
                        nc.sync.dma_start(
                            wg_t[:, do, :], wg[:, do, hg * HG : (hg + 1) * HG]
                        )
                        nc.sync.dma_start(
                            wu_t[:, do, :], wu[:, do, hg * HG : (hg + 1) * HG]
                        )

                    for t0, tn in t_chunks:
                        for jj in range(JG):
                            j = hg * JG + jj
                            pg = psA.tile([128, TA], F32, tag="pg", name="pg")[:, :tn]
                            pu = psA.tile([128, TA], F32, tag="pu", name="pu")[:, :tn]
                            for do in range(DO):
                                nc.tensor.matmul(
                                    pg,
                                    lhsT=wg_t[:, do, jj * 128 : (jj + 1) * 128],
                                    rhs=x_sb[:, do, t0 : t0 + tn],
                                    start=(do == 0),
                                    stop=(do == DO - 1),
                                )
                            for do in range(DO):
                                nc.tensor.matmul(
                                    pu,
                                    lhsT=wu_t[:, do, jj * 128 : (jj + 1) * 128],
                                    rhs=x_sb[:, do, t0 : t0 + tn],
                                    start=(do == 0),
                                    stop=(do == DO - 1),
                                )
                            # silu(pg) * pu via sigmoid, written straight into hid_sb
                            sg = hpool.tile([128, TA], F32, tag="sg", name="sg")[:, :tn]
                            nc.scalar.activation(
                                sg, pg, mybir.ActivationFunctionType.Sigmoid
                            )
                            t1 = hpool.tile([128, TA], F32, tag="t1", name="t1")[:, :tn]
                            nc.vector.tensor_mul(t1, sg, pg)
                            nc.vector.tensor_mul(hid_sb[:, j, t0 : t0 + tn], t1, pu)

            # ---- Phase B: y = hidden @ wd.T (hidden read from SBUF) ----
            DQ = 256  # d-chunk width; wd streamed once in DQ-wide quarters
            for d0 in range(0, D, DQ):
                wdh = wdpool.tile([128, HO, DQ], BF16, tag="wdh", name="wdh")
                nc.sync.dma_start(wdh[:], wd[:, :, d0 : d0 + DQ])
                for tb in range(C // 128):
                    py = psB.tile([128, DQ], F32, tag="py", name="py")
                    for j in range(HO):
                        nc.tensor.matmul(
                            py,
                            lhsT=hid_sb[:, j, tb * 128 : (tb + 1) * 128],
                            rhs=wdh[:, j, :],
                            start=(j == 0),
                            stop=(j == HO - 1),
                        )
                    y_sb = ypool.tile([128, DQ], F32, name="y_sb")
                    nc.any.tensor_copy(y_sb[:], py)
                    nc.sync.dma_start(y[tb * 128 : (tb + 1) * 128, d0 : d0 + DQ], y_sb)

    nc.compile()
    return nc


def _get_kernel(C):
    if C not in _BUILD_CACHE:
        _BUILD_CACHE[C] = _build_expert_kernel(C)
    return _BUILD_CACHE[C]


def _pack_core_inputs(x_pad, w_g_e, w_u_e, w_d_e):
    import ml_dtypes

    bf16 = ml_dtypes.bfloat16
    x_pad = x_pad.astype(bf16)
    w_g_e = w_g_e.astype(bf16)
    w_u_e = w_u_e.astype(bf16)
    w_d_e = w_d_e.astype(bf16)
    C, D = x_pad.shape
    H = w_g_e.shape[0]
    return {
        "xT": np.ascontiguousarray(x_pad.reshape(C, D // 128, 128).transpose(2, 1, 0)),
        "wg": np.ascontiguousarray(
            w_g_e.reshape(H, D // 128, 128).transpose(2, 1, 0)
        ),
        "wu": np.ascontiguousarray(
            w_u_e.reshape(H, D // 128, 128).transpose(2, 1, 0)
        ),
        "wd": np.ascontiguousarray(
            w_d_e.reshape(D, H // 128, 128).transpose(2, 1, 0)
        ),
    }


def kernel(x, w_gate, w_noise, w_g, w_u, w_d, _collect_perf=None):
    x = np.asarray(x, dtype=np.float32)
    w_gate = np.asarray(w_gate, dtype=np.float32)
    w_g = np.asarray(w_g, dtype=np.float32)
    w_u = np.asarray(w_u, dtype=np.float32)
    w_d = np.asarray(w_d, dtype=np.float32)

    B, S, D = x.shape
    T = B * S
    xf = x.reshape(T, D)

    # ---- Gating (host; decides the token dispatch) ----
    logits = xf @ w_gate.T  # (T, E) fp32
    rows = np.arange(T)
    top1 = np.argmax(logits, axis=1)
    l1 = logits[rows, top1]
    masked = logits.copy()
    masked[rows, top1] = -np.inf
    top2 = np.argmax(masked, axis=1)
    l2 = logits[rows, top2]
    # softmax over the two top logits (l1 >= l2)
    e2 = np.exp((l2 - l1).astype(np.float64))
    g1 = (1.0 / (1.0 + e2)).astype(np.float32)
    g2 = (e2 / (1.0 + e2)).astype(np.float32)

    # aux loss: E * sum(mean(gates) * mean(softmax(logits)))
    lg64 = logits.astype(np.float64)
    lg64 -= lg64.max(axis=1, keepdims=True)
    p = np.exp(lg64)
    p /= p.sum(axis=1, keepdims=True)
    P = p.mean(axis=0)
    f = np.zeros(N_EXPERTS, dtype=np.float64)
    np.add.at(f, top1, g1.astype(np.float64))
    np.add.at(f, top2, g2.astype(np.float64))
    f /= T
    aux_loss = np.float32(N_EXPERTS * np.sum(f * P))

    # ---- Token dispatch (host all-to-all) ----
    idxs, gates_e = [], []
    for e in range(N_EXPERTS):
        m1 = top1 == e
        m2 = top2 == e
        idx = np.nonzero(m1 | m2)[0]
        ge = np.where(m1[idx], g1[idx], g2[idx]).astype(np.float32)
        idxs.append(idx)
        gates_e.append(ge)

    max_cnt = max(len(i) for i in idxs)
    C = max(128, ((max_cnt + 127) // 128) * 128)
    nc = _get_kernel(C)

    in_maps = []
    for e in range(N_EXPERTS):
        idx = idxs[e]
        x_pad = np.zeros((C, D), np.float32)
        x_pad[: len(idx)] = xf[idx]
        in_maps.append(_pack_core_inputs(x_pad, w_g[e], w_u[e], w_d[e]))

    trace = bool(int(os.environ.get("MOE_KERNEL_TRACE", "0")))
    res = run_bass_kernel_spmd(
        nc,
        in_maps,
        list(range(N_EXPERTS)),
        trace=trace,
        trace_cores=list(range(N_EXPERTS)) if trace else None,
    )
    if _collect_perf is not None:
        _collect_perf.append(res)

    # ---- Combine (host scatter-add of gated expert outputs) ----
    out = np.zeros((T, D), np.float32)
    for e in range(N_EXPERTS):
        idx = idxs[e]
        if len(idx) == 0:
            continue
        ye = np.asarray(res.results[e]["y"])[: len(idx)]
        out[idx] += gates_e[e][:, None] * ye

    return out.reshape(B, S, D), aux_loss
